# revision 30
# baseline (speedup 1.0000x reference)
"""MLA attention kernel for Trainium2, 8 NeuronCores.

Sharding: core = (batch b in {0,1}) x (head-group hg in {0..3}, 4 heads each).
The down-projections are additionally sharded across the 4 cores of a batch
group (core hg computes 1/4 of c_kv^T and of q_lat^T) and all-gathered via
gpsimd collective_compute over DRAM bounce buffers; the c_kv gather overlaps
the q_lat partial compute, the q_lat gather overlaps U2. Each core then runs
its 4 heads' up-projections + RoPE + causal attention + the partial o_proj
contribution; the host sums the 4 partial outputs per batch.

dtypes: all matmuls run at 1 PE cycle/row. The QK path (up-proj outputs,
scores) is float32r; hidden/latents/up-weights and the probs/V/o_proj side are
bf16 (PSUM always accumulates fp32). Measured rel err ~4e-3 vs the fp32
reference (gate 2e-2).

Layouts (all matmuls transpose-free; host passes hidden[b]^T):
  q_lat^T [768,S] -> q_nope^T/q_rope^T [d,S] (d-major)   (scores lhsT/rhs)
  c_kv^T [512,S]  -> k_nope^T/k_rope^T [d,S], V [S,dv]   (token-major V = PV lhsT)
  scores^T [k,q] -> exp -> probs^T (PV rhs), out^T [dv,q] -> o_proj lhsT.
Rope dims are pair-packed: two heads per [128,S] tile.

Softmax: no max-subtraction (|score| small); denominator accumulated on DVE
(bf16 adds of the exp tiles) + one ones-matmul per head; reciprocal [1,512] on
DVE; broadcast across partitions via gpsimd.partition_broadcast (no PE
dependency on the normalization chain). Score matmuls are emitted through a
flat cursor 2 tiles ahead of their consumers, across head and q-block
boundaries, to keep the PE dense (p-state) while exp/pv/softmax-tail drain.
Causal handling: fully-masked k/q tile pairs are skipped; diagonal tiles get
the mask added in-place on PSUM by the DVE before the exp.

Queue discipline: gpsimd issues only the collectives (a blocking CC must not
head-of-line DMA loads); gather reloads go on the scalar hwdge queue so the
sync queue keeps prefetching weights while the reloads park on the CC
semaphore.
"""
import sys

sys.path.insert(0, "/opt/trn_rl_repo")

import numpy as np
import ml_dtypes
import concourse.bass as bass
import concourse.bacc as bacc
import concourse.tile as tile
from concourse import mybir
from concourse.bass_utils import run_bass_kernel_spmd

FP = mybir.dt.float32
BF = mybir.dt.bfloat16
S = 2048
HID = 2048
H = 16
DN = 128
DR = 64
DV = 128
QL = 768
KVL = 512
ROPE_BASE = 10000.0
SCALE = (DN + DR) ** -0.5
NEG = -1e9
NCORES = 8
HPC = 4  # heads per core
P = 128
NB = S // 512  # 4 query/key column blocks of 512
KT = S // P  # 16 token tiles of 128
QLT = QL // P  # 6
KVT = KVL // P  # 4

_cache = {}


def _build(variant):
    """variant: 'causal' (on-chip mask + tile skipping), 'zeros' (no mask),
    'generic' (mask^T DMA'd from DRAM, all tiles)."""
    nc = bacc.Bacc()

    # All matmuls in fp32r: 1 PE cycle/row (vs 4 for fp32) when the output
    # free dim is >=256, which holds for every matmul here (free=512+).
    R = mybir.dt.float32r

    def mm(out, lhsT, rhs, **kw):
        if lhsT.dtype == FP:
            lhsT = lhsT.bitcast(R)
        if rhs.dtype == FP:
            rhs = rhs.bitcast(R)
        nc.tensor.matmul(out, lhsT, rhs, **kw)

    hidT = nc.dram_tensor("hidT", [HID, S], BF, kind="ExternalInput")
    w_qd_s = nc.dram_tensor("w_qd_s", [HID, QL // 4], BF, kind="ExternalInput")
    w_kvd_s = nc.dram_tensor("w_kvd_s", [HID, KVL // 4], BF,
                             kind="ExternalInput")
    w_qu = nc.dram_tensor("w_qu", [QL, HPC * DN], BF, kind="ExternalInput")
    w_qr = nc.dram_tensor("w_qr", [QL, HPC * DR], BF, kind="ExternalInput")
    w_ku = nc.dram_tensor("w_ku", [KVL, HPC * DN], BF, kind="ExternalInput")
    w_kr = nc.dram_tensor("w_kr", [KVL, HPC * DR], BF, kind="ExternalInput")
    w_vu = nc.dram_tensor("w_vu", [KVL, HPC * DV], BF, kind="ExternalInput")
    w_o = nc.dram_tensor("w_o", [HPC * DV, HID], BF, kind="ExternalInput")
    cs = nc.dram_tensor("cs", [128, 2 * S], R, kind="ExternalInput")
    if variant == "generic":
        maskT = nc.dram_tensor("maskT", [S, S], FP, kind="ExternalInput")
    o_out = nc.dram_tensor("o", [S, HID], FP, kind="ExternalOutput")

    def down_part(tc, w_dram, widths, out_tiles, scale):
        # Partial down-projection: out_tiles[i] ([widths[i], S]) = slice of
        # (w^T @ hidT). One [128,2048] bf16 hidT load per contraction tile
        # (issued on alternating queues); all 4 n-blocks' psum chains run per
        # load, so the PE sees len(widths)*4 matmuls per DMA.
        with tc.tile_pool(name="wdp", bufs=1) as wp, \
             tc.tile_pool(name="rhsd", bufs=3) as rp, \
             tc.tile_pool(name="psdp", bufs=1, space="PSUM") as pp:
            tw = sum(widths)
            wt = [wp.tile([P, tw], BF, name=f"wd{k}", tag=f"wd{k}")
                  for k in range(KT)]
            for k in range(KT):
                nc.sync.dma_start(out=wt[k][:], in_=w_dram[k * P:(k + 1) * P, :])
            pss = [[pp.tile([w, 512], FP, name=f"pd{i}_{n}", tag=f"pd{i}_{n}")
                    for n in range(NB)] for i, w in enumerate(widths)]
            for k in range(KT):
                r = rp.tile([P, S], BF, name="rd", tag="rd")
                nc.sync.dma_start(out=r[:], in_=hidT[k * P:(k + 1) * P, :])
                for n in range(NB):
                    off = 0
                    for i, w in enumerate(widths):
                        mm(pss[i][n][:], wt[k][:, off:off + w],
                           r[:, n * 512:(n + 1) * 512],
                           start=(k == 0), stop=(k == KT - 1))
                        off += w
            for i, w in enumerate(widths):
                for n in range(NB):
                    if scale is not None:
                        nc.scalar.activation(
                            out_tiles[i][:, n * 512:(n + 1) * 512],
                            pss[i][n][:], mybir.ActivationFunctionType.Copy,
                            scale=float(scale))
                    elif n % 2 == 0:
                        nc.scalar.copy(out_tiles[i][:, n * 512:(n + 1) * 512],
                                       pss[i][n][:])
                    else:
                        nc.vector.tensor_copy(
                            out_tiles[i][:, n * 512:(n + 1) * 512],
                            pss[i][n][:])

    def rope_block(tc, x, csp, tp, n):
        # in-place rope on x[:, n*512:(n+1)*512] of a pair-packed [128,S] tile
        cob = csp.tile([P, 512], R, name="cob", tag="cob")
        nc.sync.dma_start(out=cob[:], in_=cs[:, n * 512:(n + 1) * 512])
        snb = csp.tile([P, 512], R, name="snb", tag="snb")
        nc.sync.dma_start(out=snb[:], in_=cs[:, S + n * 512:S + (n + 1) * 512])
        xb = x[:, n * 512:(n + 1) * 512]
        t2 = tp.tile([P, 512], R, name="t2", tag="t2")
        for q in range(4):
            src = (q // 2) * 64 + (32 if q % 2 == 0 else 0)
            # partition-shifted copy (1-input; shifted 2-input TT is rejected
            # by the compiler: base partitions must match)
            nc.vector.tensor_copy(t2[q * 32:(q + 1) * 32], xb[src:src + 32])
        nc.vector.tensor_tensor(t2[:], t2[:], snb[:], mybir.AluOpType.mult)
        nc.vector.tensor_tensor(xb, xb, cob[:], mybir.AluOpType.mult)
        nc.vector.tensor_tensor(xb, xb, t2[:], mybir.AluOpType.add)

    def up_proj(tc, wp, w_sb, kt, rhs_tiles, out_tile, h_cols, n, ev):
        # out_tile[:, nb] = w_sb[:, h_cols]^T @ rhs, contraction kt tiles
        ps = upp[0].tile([P, 512], FP, name="psu", tag="psu")
        for k in range(kt):
            mm(ps[:], w_sb[k][:, h_cols], rhs_tiles[k][:, n * 512:(n + 1) * 512],
                             start=(k == 0), stop=(k == kt - 1))
        nc.scalar.copy(out_tile[:, n * 512:(n + 1) * 512], ps[:])

    with tile.TileContext(nc) as tc:
        with tc.tile_pool(name="kvout", bufs=1, side="right") as kvp:
            k_nope = [kvp.tile([P, S], R, name=f"kn{h}", tag=f"kn{h}")
                      for h in range(HPC)]
            k_rope = [kvp.tile([P, S], R, name=f"kr{p}", tag=f"kr{p}")
                      for p in range(HPC // 2)]
            v4 = [kvp.tile([P, HPC * DV], BF, name=f"v{t}", tag=f"v{t}")
                  for t in range(KT)]

            # ---- D2'/D1': sharded down-projections + all-gather ----
            # Core r of each 4-core batch group computes 1/4 of c_kv^T and
            # 1/4 of q_lat^T; both are all-gathered (DRAM bounce) while the
            # other partial / U2 runs on the PE.
            REP = [[0, 1, 2, 3], [4, 5, 6, 7]]
            dgp = tc.alloc_tile_pool(name="dgather", bufs=1, space="DRAM")
            ckv_in = dgp.tile([P, S], BF, name="ckv_in", tag="ckv_in")
            ckv_all = dgp.tile([KVL, S], BF, name="ckv_all", tag="ckv_all")
            ql_in = dgp.tile([QL // 4, S], BF, name="ql_in", tag="ql_in")
            ql_all = dgp.tile([QL, S], BF, name="ql_all", tag="ql_all")
            dpp = tc.alloc_tile_pool(name="dpart", bufs=1)
            ckv_part = dpp.tile([P, S], BF, name="ckv_part", tag="ckv_part")
            ql_part = [dpp.tile([P, S], BF, name="qlp0", tag="qlp0"),
                       dpp.tile([64, S], BF, name="qlp1", tag="qlp1")]

            down_part(tc, w_kvd_s, [P], [ckv_part], None)
            nc.sync.dma_start(out=ckv_in[:], in_=ckv_part[:])
            nc.gpsimd.collective_compute(
                "AllGather", mybir.AluOpType.bypass, replica_groups=REP,
                ins=[ckv_in[:].opt()], outs=[ckv_all[:].opt()])
            down_part(tc, w_qd_s, [P, 64], ql_part, SCALE)
            nc.sync.dma_start(out=ql_in[0:P], in_=ql_part[0][:])
            nc.sync.dma_start(out=ql_in[P:P + 64], in_=ql_part[1][:])
            nc.gpsimd.collective_compute(
                "AllGather", mybir.AluOpType.bypass, replica_groups=REP,
                ins=[ql_in[:].opt()], outs=[ql_all[:].opt()])
            dpp.release()

            cp_ = tc.alloc_tile_pool(name="ckvp", bufs=1)
            if True:
                c_kv = [cp_.tile([P, S], BF, name=f"ckv{m}", tag=f"ckv{m}")
                        for m in range(KVT)]
                # scalar hwdge queue: a reload parked on the CC semaphore
                # must not block the weight prefetches on the sync queue.
                # n-major chunks: U2's first column block only waits for the
                # first 4 chunks instead of the whole gather.
                for n in range(NB):
                    for m in range(KVT):
                        nc.scalar.dma_start(
                            out=c_kv[m][:, n * 512:(n + 1) * 512],
                            in_=ckv_all[m * P:(m + 1) * P,
                                        n * 512:(n + 1) * 512])

                # ---- U2: k-ups + rope-k + V4 ----
                with tc.tile_pool(name="wku", bufs=1) as wp, \
                     tc.tile_pool(name="csp2", bufs=4) as csp, \
                     tc.tile_pool(name="tp2", bufs=2) as tp, \
                     tc.tile_pool(name="psu2", bufs=3, space="PSUM") as pu:
                    upp = [pu]
                    ku = [wp.tile([P, HPC * DN], BF, name=f"wku{k}", tag=f"wku{k}")
                          for k in range(KVT)]
                    kr = [wp.tile([P, HPC * DR], BF, name=f"wkr{k}", tag=f"wkr{k}")
                          for k in range(KVT)]
                    vu = [wp.tile([P, HPC * DV], BF, name=f"wvu{k}", tag=f"wvu{k}")
                          for k in range(KVT)]
                    for k in range(KVT):
                        nc.sync.dma_start(out=ku[k][:], in_=w_ku[k * P:(k + 1) * P, :])
                        nc.sync.dma_start(out=kr[k][:], in_=w_kr[k * P:(k + 1) * P, :])
                        nc.sync.dma_start(out=vu[k][:], in_=w_vu[k * P:(k + 1) * P, :])
                    for n in range(NB):
                        for h in range(HPC):
                            up_proj(tc, pu, ku, KVT, c_kv, k_nope[h],
                                    slice(h * DN, (h + 1) * DN), n, h % 2)
                        for p in range(HPC // 2):
                            up_proj(tc, pu, kr, KVT, c_kv, k_rope[p],
                                    slice(p * 2 * DR, (p + 1) * 2 * DR), n, p % 2)
                            rope_block(tc, k_rope[p], csp, tp, n)
                    for t in range(KT):
                        ps = pu.tile([P, HPC * DV], FP, name="psv", tag="psv")
                        for k in range(KVT):
                            mm(ps[:], c_kv[k][:, t * P:(t + 1) * P],
                                             vu[k][:], start=(k == 0),
                                             stop=(k == KVT - 1))
                        if t % 2 == 0:
                            nc.scalar.copy(v4[t][:], ps[:])
                        else:
                            nc.vector.tensor_copy(v4[t][:], ps[:])

            # ---- D1 reload: gathered q_lat^T (already scaled) ----
            cp_.release()
            qlp = tc.alloc_tile_pool(name="qlatp", bufs=1)
            q_lat = [qlp.tile([P, S], BF, name=f"ql{m}", tag=f"ql{m}")
                     for m in range(QLT)]
            # sync queue: it is idle during U2, so these issue as soon as
            # the q_lat gather lands instead of queueing behind U2's evicts.
            for n in range(NB):
                for m in range(QLT):
                    nc.sync.dma_start(
                        out=q_lat[m][:, n * 512:(n + 1) * 512],
                        in_=ql_all[m * P:(m + 1) * P, n * 512:(n + 1) * 512])

            with tc.tile_pool(name="qout", bufs=1, side="right") as qp:
                q_nope = [qp.tile([P, S], R, name=f"qn{h}", tag=f"qn{h}")
                          for h in range(HPC)]
                q_rope = [qp.tile([P, S], R, name=f"qr{p}", tag=f"qr{p}")
                          for p in range(HPC // 2)]

                # ---- U1: q-ups + rope-q ----
                with tc.tile_pool(name="wqup", bufs=1) as wp, \
                     tc.tile_pool(name="csp1", bufs=2) as csp, \
                     tc.tile_pool(name="tp1", bufs=1) as tp, \
                     tc.tile_pool(name="psu1", bufs=4, space="PSUM") as pu:
                    upp = [pu]
                    wu = [wp.tile([P, HPC * DN], BF, name=f"wqu{k}", tag=f"wqu{k}")
                          for k in range(QLT)]
                    wr = [wp.tile([P, HPC * DR], BF, name=f"wqr{k}", tag=f"wqr{k}")
                          for k in range(QLT)]
                    for k in range(QLT):
                        nc.sync.dma_start(out=wu[k][:], in_=w_qu[k * P:(k + 1) * P, :])
                        nc.sync.dma_start(out=wr[k][:], in_=w_qr[k * P:(k + 1) * P, :])
                    for n in range(NB):
                        for h in range(HPC):
                            up_proj(tc, pu, wu, QLT, q_lat, q_nope[h],
                                    slice(h * DN, (h + 1) * DN), n, h % 2)
                        for p in range(HPC // 2):
                            up_proj(tc, pu, wr, QLT, q_lat, q_rope[p],
                                    slice(p * 2 * DR, (p + 1) * 2 * DR), n, p % 2)
                            rope_block(tc, q_rope[p], csp, tp, n)
                qlp.release()

                # ---- ATT + o_proj per q-block ----
                with tc.tile_pool(name="att_c", bufs=1) as cp, \
                     tc.tile_pool(name="dnsb", bufs=2) as dnp, \
                     tc.tile_pool(name="probs", bufs=4) as prp, \
                     tc.tile_pool(name="attn", bufs=5) as atp, \
                     tc.tile_pool(name="osb", bufs=2) as osp, \
                     tc.tile_pool(name="rdn", bufs=2) as rdp, \
                     tc.tile_pool(name="rbp", bufs=2) as rbp, \
                     tc.tile_pool(name="ps_s", bufs=3, space="PSUM") as ps_s, \
                     tc.tile_pool(name="ps_pv", bufs=2, space="PSUM") as ps_pv, \
                     tc.tile_pool(name="ps_den", bufs=2, space="PSUM") as ps_den, \
                     tc.tile_pool(name="ps_o", bufs=1, space="PSUM") as ps_o:
                    wo = [cp.tile([P, HID], BF, name=f"wo{k}", tag=f"wo{k}")
                          for k in range(HPC)]
                    for k in range(HPC):
                        nc.sync.dma_start(out=wo[k][:], in_=w_o[k * P:(k + 1) * P, :])
                    ones_k = cp.tile([P, 2], BF, name="ones_k", tag="ones_k")
                    nc.vector.memset(ones_k[:], 1.0)
                    if variant == "causal":
                        msk = cp.tile([P, 896], FP, name="msk", tag="msk")
                        nc.gpsimd.memset(msk[:], 0.0)
                        nc.gpsimd.affine_select(
                            out=msk[:], in_=msk[:],
                            compare_op=mybir.AluOpType.is_ge,
                            fill=NEG, base=-384,
                            pattern=[[1, 896]], channel_multiplier=-1)
                    if variant == "generic":
                        mrp = tc.alloc_tile_pool(name="mrhs", bufs=18)

                    # Scores are pipelined through a flat emission cursor that
                    # stays 2 tiles ahead of consumption, across head and
                    # q-block boundaries: while the softmax tail (den matmul ->
                    # reciprocal -> broadcast -> normalize) and o_proj of one
                    # block drain, the PE is already streaming the next head's
                    # score matmuls.
                    def nkt_of(jj):
                        return 4 * (jj + 1) if variant == "causal" else KT

                    seq = [(jj, h, ki) for jj in range(NB) for h in range(HPC)
                           for ki in range(nkt_of(jj))]
                    pos_of = {t: i for i, t in enumerate(seq)}

                    def emit_ss(jj, h, ki):
                        pp_, hh = h // 2, (h % 2) * DR
                        ss = ps_s.tile([P, 512], FP)
                        mm(ss[:], k_nope[h][:, ki * P:(ki + 1) * P],
                           q_nope[h][:, jj * 512:(jj + 1) * 512],
                           start=True, stop=False)
                        mm(ss[:],
                           k_rope[pp_][hh:hh + DR, ki * P:(ki + 1) * P],
                           q_rope[pp_][hh:hh + DR, jj * 512:(jj + 1) * 512],
                           start=False, stop=True)
                        return ss

                    emitted = {}
                    cursor = [0]

                    def ensure_ss(idx):
                        while cursor[0] <= min(idx, len(seq) - 1):
                            t = seq[cursor[0]]
                            emitted[t] = emit_ss(*t)
                            cursor[0] += 1

                    for j in range(NB):
                        nkt = nkt_of(j)
                        mts = []
                        if variant == "generic":
                            for ki in range(KT):
                                mt = mrp.tile([P, 512], FP, name="mrhs", tag="mrhs")
                                nc.sync.dma_start(
                                    out=mt[:],
                                    in_=maskT[ki * P:(ki + 1) * P,
                                              j * 512:(j + 1) * 512])
                                mts.append(mt)
                        attn_sb = []
                        for h in range(HPC):
                            pv = ps_pv.tile([P, 512], FP)
                            den_acc = dnp.tile([P, 512], BF, name="den_acc",
                                               tag="den_acc")
                            for ki in range(nkt):
                                ensure_ss(pos_of[(j, h, ki)] + 2)
                                ss = emitted.pop((j, h, ki))
                                pr = prp.tile([P, 512], BF, name="pr", tag="pr")
                                off = 128 * ki - 512 * j
                                if variant == "causal" and off >= 0:
                                    c0 = 384 - off
                                    nc.vector.tensor_tensor(
                                        ss[:], ss[:], msk[:, c0:c0 + 512],
                                        mybir.AluOpType.add)
                                elif variant == "generic":
                                    nc.vector.tensor_tensor(
                                        ss[:], ss[:], mts[ki][:],
                                        mybir.AluOpType.add)
                                nc.scalar.activation(
                                    pr[:], ss[:],
                                    mybir.ActivationFunctionType.Exp)
                                mm(pv[:], v4[ki][:, h * DV:(h + 1) * DV],
                                   pr[:], start=(ki == 0),
                                   stop=(ki == nkt - 1))
                                if ki == 0:
                                    nc.vector.tensor_copy(den_acc[:], pr[:])
                                else:
                                    nc.vector.tensor_tensor(
                                        den_acc[:], den_acc[:], pr[:],
                                        mybir.AluOpType.add)
                            den = ps_den.tile([2, 512], FP)
                            mm(den[:], ones_k[:], den_acc[:],
                               start=True, stop=True)
                            rden = rdp.tile([1, 512], FP, name="rden",
                                            tag="rden")
                            nc.vector.reciprocal(rden[:], den[0:1])
                            rb = rbp.tile([P, 512], FP, name="rb", tag="rb")
                            nc.gpsimd.partition_broadcast(rb[:], rden[:])
                            at = atp.tile([P, 512], BF, name="at", tag="at")
                            nc.vector.tensor_tensor(at[:], pv[:], rb[:],
                                                    mybir.AluOpType.mult)
                            attn_sb.append(at)
                        for t in range(4):
                            ob = osp.tile([P, HID], FP, name="ob", tag="ob")
                            for nn in range(NB):
                                po = ps_o.tile([P, 512], FP)
                                for kk in range(HPC):
                                    mm(
                                        po[:], attn_sb[kk][:, t * P:(t + 1) * P],
                                        wo[kk][:, nn * 512:(nn + 1) * 512],
                                        start=(kk == 0), stop=(kk == HPC - 1))
                                if nn % 2 == 0:
                                    nc.scalar.copy(ob[:, nn * 512:(nn + 1) * 512],
                                                   po[:])
                                else:
                                    nc.vector.tensor_copy(
                                        ob[:, nn * 512:(nn + 1) * 512], po[:])
                            nc.sync.dma_start(
                                out=o_out[(j * 4 + t) * P:(j * 4 + t + 1) * P, :],
                                in_=ob[:])
                    if variant == "generic":
                        mrp.release()

    nc.compile()
    return nc


def _get(variant):
    if variant not in _cache:
        _cache[variant] = _build(variant)
    return _cache[variant]


def _host_prep(inputs):
    hs = np.ascontiguousarray(inputs["hidden_states"], dtype=np.float32)
    mask = np.asarray(inputs["attention_mask"], dtype=np.float32)
    pos = np.asarray(inputs["position_ids"])
    B = hs.shape[0]

    causal = np.where(np.tril(np.ones((S, S), dtype=bool)), np.float32(0.0),
                      np.float32(NEG))
    variant = "causal"
    for b in range(B):
        if not np.array_equal(mask[b, 0], causal):
            variant = "zeros" if not mask.any() else "generic"
            break

    inv_freq = (1.0 / (ROPE_BASE ** (np.arange(0, DR, 2, dtype=np.float32) / DR)))
    css = []
    for b in range(B):
        t = pos[b].astype(np.float32)
        freqs = t[:, None] * inv_freq[None, :]  # [S, 32]
        cf = np.cos(freqs).T  # [32, S]
        sf = np.sin(freqs).T
        cs = np.empty((128, 2 * S), dtype=np.float32)
        for q in range(4):
            cs[q * 32:(q + 1) * 32, :S] = cf
            cs[q * 32:(q + 1) * 32, S:] = sf if q % 2 else -sf
        css.append(np.ascontiguousarray(cs))
    return hs, mask, css, variant


def kernel(**inputs):
    hs, mask, css, variant = _host_prep(inputs)
    nc = _get(variant)

    w_qd = np.asarray(inputs["W_q_down"], dtype=np.float32)
    w_kvd = np.asarray(inputs["W_kv_down"], dtype=np.float32)
    W_qu = np.asarray(inputs["W_q_up"], dtype=np.float32)
    W_qr = np.asarray(inputs["W_q_rope"], dtype=np.float32)
    W_ku = np.asarray(inputs["W_k_up"], dtype=np.float32)
    W_kr = np.asarray(inputs["W_k_rope"], dtype=np.float32)
    W_vu = np.asarray(inputs["W_v_up"], dtype=np.float32)
    W_o = np.asarray(inputs["W_o"], dtype=np.float32)

    hidT = [np.ascontiguousarray(hs[b].T).astype(ml_dtypes.bfloat16)
            for b in range(2)]
    maskT = [np.ascontiguousarray(mask[b, 0].T) for b in range(2)] \
        if variant == "generic" else None

    in_maps = []
    for core in range(NCORES):
        b, hg = divmod(core, NCORES // 2)
        m = {
            "hidT": hidT[b],
            "w_qd_s": np.ascontiguousarray(
                w_qd[:, hg * (QL // 4):(hg + 1) * (QL // 4)]).astype(
                    ml_dtypes.bfloat16),
            "w_kvd_s": np.ascontiguousarray(
                w_kvd[:, hg * (KVL // 4):(hg + 1) * (KVL // 4)]).astype(
                    ml_dtypes.bfloat16),
            "w_qu": np.ascontiguousarray(
                W_qu[:, hg * HPC * DN:(hg + 1) * HPC * DN]).astype(
                    ml_dtypes.bfloat16),
            "w_qr": np.ascontiguousarray(
                W_qr[:, hg * HPC * DR:(hg + 1) * HPC * DR]).astype(
                    ml_dtypes.bfloat16),
            "w_ku": np.ascontiguousarray(
                W_ku[:, hg * HPC * DN:(hg + 1) * HPC * DN]).astype(
                    ml_dtypes.bfloat16),
            "w_kr": np.ascontiguousarray(
                W_kr[:, hg * HPC * DR:(hg + 1) * HPC * DR]).astype(
                    ml_dtypes.bfloat16),
            "w_vu": np.ascontiguousarray(
                W_vu[:, hg * HPC * DV:(hg + 1) * HPC * DV]).astype(
                    ml_dtypes.bfloat16),
            "w_o": np.ascontiguousarray(
                W_o[hg * HPC * DV:(hg + 1) * HPC * DV, :]).astype(
                    ml_dtypes.bfloat16),
            "cs": css[b],
        }
        if maskT is not None:
            m["maskT"] = maskT[b]
        in_maps.append(m)

    global _last_in_maps, _last_nc
    _last_in_maps, _last_nc = in_maps, nc
    res = run_bass_kernel_spmd(nc, in_maps, core_ids=list(range(NCORES)))
    out = np.zeros((2, S, HID), dtype=np.float32)
    for core in range(NCORES):
        b = core // (NCORES // 2)
        out[b] += res.results[core]["o"]
    return out



# revision 31
# speedup vs baseline: 1.0020x; 1.0020x over previous
"""MLA attention kernel for Trainium2, 8 NeuronCores.

Sharding: core = (batch b in {0,1}) x (head-group hg in {0..3}, 4 heads each).
The down-projections are additionally sharded across the 4 cores of a batch
group (core hg computes 1/4 of c_kv^T and of q_lat^T) and all-gathered via
gpsimd collective_compute over DRAM bounce buffers; the c_kv gather overlaps
the q_lat partial compute, the q_lat gather overlaps U2. Each core then runs
its 4 heads' up-projections + RoPE + causal attention + the partial o_proj
contribution; the host sums the 4 partial outputs per batch.

dtypes: all matmuls run at 1 PE cycle/row. The QK path (up-proj outputs,
scores) is float32r; hidden/latents/up-weights and the probs/V/o_proj side are
bf16 (PSUM always accumulates fp32). Measured rel err ~4e-3 vs the fp32
reference (gate 2e-2).

Layouts (all matmuls transpose-free; host passes hidden[b]^T):
  q_lat^T [768,S] -> q_nope^T/q_rope^T [d,S] (d-major)   (scores lhsT/rhs)
  c_kv^T [512,S]  -> k_nope^T/k_rope^T [d,S], V [S,dv]   (token-major V = PV lhsT)
  scores^T [k,q] -> exp -> probs^T (PV rhs), out^T [dv,q] -> o_proj lhsT.
Rope dims are pair-packed: two heads per [128,S] tile.

Softmax: no max-subtraction (|score| small); denominator accumulated on DVE
(bf16 adds of the exp tiles) + one ones-matmul per head; reciprocal [1,512] on
DVE; broadcast across partitions via gpsimd.partition_broadcast (no PE
dependency on the normalization chain). Score matmuls are emitted through a
flat cursor 2 tiles ahead of their consumers, across head and q-block
boundaries, to keep the PE dense (p-state) while exp/pv/softmax-tail drain.
Causal handling: fully-masked k/q tile pairs are skipped; diagonal tiles get
the mask added in-place on PSUM by the DVE before the exp.

Queue discipline: gpsimd issues only the collectives (a blocking CC must not
head-of-line DMA loads); gather reloads go on the scalar hwdge queue so the
sync queue keeps prefetching weights while the reloads park on the CC
semaphore.
"""
import sys

sys.path.insert(0, "/opt/trn_rl_repo")

import numpy as np
import ml_dtypes
import concourse.bass as bass
import concourse.bacc as bacc
import concourse.tile as tile
from concourse import mybir
from concourse.bass_utils import run_bass_kernel_spmd

FP = mybir.dt.float32
BF = mybir.dt.bfloat16
S = 2048
HID = 2048
H = 16
DN = 128
DR = 64
DV = 128
QL = 768
KVL = 512
ROPE_BASE = 10000.0
SCALE = (DN + DR) ** -0.5
NEG = -1e9
NCORES = 8
HPC = 4  # heads per core
P = 128
NB = S // 512  # 4 query/key column blocks of 512
KT = S // P  # 16 token tiles of 128
QLT = QL // P  # 6
KVT = KVL // P  # 4

_cache = {}


def _build(variant):
    """variant: 'causal' (on-chip mask + tile skipping), 'zeros' (no mask),
    'generic' (mask^T DMA'd from DRAM, all tiles)."""
    nc = bacc.Bacc()

    # All matmuls in fp32r: 1 PE cycle/row (vs 4 for fp32) when the output
    # free dim is >=256, which holds for every matmul here (free=512+).
    R = mybir.dt.float32r

    def mm(out, lhsT, rhs, **kw):
        if lhsT.dtype == FP:
            lhsT = lhsT.bitcast(R)
        if rhs.dtype == FP:
            rhs = rhs.bitcast(R)
        nc.tensor.matmul(out, lhsT, rhs, **kw)

    hidT = nc.dram_tensor("hidT", [HID, S], BF, kind="ExternalInput")
    w_qd_s = nc.dram_tensor("w_qd_s", [HID, QL // 4], BF, kind="ExternalInput")
    w_kvd_s = nc.dram_tensor("w_kvd_s", [HID, KVL // 4], BF,
                             kind="ExternalInput")
    w_qu = nc.dram_tensor("w_qu", [QL, HPC * DN], BF, kind="ExternalInput")
    w_qr = nc.dram_tensor("w_qr", [QL, HPC * DR], BF, kind="ExternalInput")
    w_ku = nc.dram_tensor("w_ku", [KVL, HPC * DN], BF, kind="ExternalInput")
    w_kr = nc.dram_tensor("w_kr", [KVL, HPC * DR], BF, kind="ExternalInput")
    w_vu = nc.dram_tensor("w_vu", [KVL, HPC * DV], BF, kind="ExternalInput")
    w_o = nc.dram_tensor("w_o", [HPC * DV, HID], BF, kind="ExternalInput")
    cs = nc.dram_tensor("cs", [128, 2 * S], R, kind="ExternalInput")
    if variant == "generic":
        maskT = nc.dram_tensor("maskT", [S, S], FP, kind="ExternalInput")
    o_out = nc.dram_tensor("o", [S, HID], FP, kind="ExternalOutput")

    def down_part(tc, w_dram, widths, out_tiles, scale):
        # Partial down-projection: out_tiles[i] ([widths[i], S]) = slice of
        # (w^T @ hidT). One [128,2048] bf16 hidT load per contraction tile
        # (issued on alternating queues); all 4 n-blocks' psum chains run per
        # load, so the PE sees len(widths)*4 matmuls per DMA.
        with tc.tile_pool(name="wdp", bufs=1) as wp, \
             tc.tile_pool(name="rhsd", bufs=3) as rp, \
             tc.tile_pool(name="psdp", bufs=1, space="PSUM") as pp:
            tw = sum(widths)
            wt = [wp.tile([P, tw], BF, name=f"wd{k}", tag=f"wd{k}")
                  for k in range(KT)]
            for k in range(KT):
                nc.sync.dma_start(out=wt[k][:], in_=w_dram[k * P:(k + 1) * P, :])
            pss = [[pp.tile([w, 512], FP, name=f"pd{i}_{n}", tag=f"pd{i}_{n}")
                    for n in range(NB)] for i, w in enumerate(widths)]
            for k in range(KT):
                r = rp.tile([P, S], BF, name="rd", tag="rd")
                nc.sync.dma_start(out=r[:], in_=hidT[k * P:(k + 1) * P, :])
                for n in range(NB):
                    off = 0
                    for i, w in enumerate(widths):
                        mm(pss[i][n][:], wt[k][:, off:off + w],
                           r[:, n * 512:(n + 1) * 512],
                           start=(k == 0), stop=(k == KT - 1))
                        off += w
            for i, w in enumerate(widths):
                for n in range(NB):
                    if scale is not None:
                        nc.scalar.activation(
                            out_tiles[i][:, n * 512:(n + 1) * 512],
                            pss[i][n][:], mybir.ActivationFunctionType.Copy,
                            scale=float(scale))
                    elif n % 2 == 0:
                        nc.scalar.copy(out_tiles[i][:, n * 512:(n + 1) * 512],
                                       pss[i][n][:])
                    else:
                        nc.vector.tensor_copy(
                            out_tiles[i][:, n * 512:(n + 1) * 512],
                            pss[i][n][:])

    def rope_block(tc, x, csp, tp, n):
        # in-place rope on x[:, n*512:(n+1)*512] of a pair-packed [128,S] tile
        cob = csp.tile([P, 512], R, name="cob", tag="cob")
        nc.sync.dma_start(out=cob[:], in_=cs[:, n * 512:(n + 1) * 512])
        snb = csp.tile([P, 512], R, name="snb", tag="snb")
        nc.sync.dma_start(out=snb[:], in_=cs[:, S + n * 512:S + (n + 1) * 512])
        xb = x[:, n * 512:(n + 1) * 512]
        t2 = tp.tile([P, 512], R, name="t2", tag="t2")
        for q in range(4):
            src = (q // 2) * 64 + (32 if q % 2 == 0 else 0)
            # partition-shifted copy (1-input; shifted 2-input TT is rejected
            # by the compiler: base partitions must match)
            nc.vector.tensor_copy(t2[q * 32:(q + 1) * 32], xb[src:src + 32])
        nc.vector.tensor_tensor(t2[:], t2[:], snb[:], mybir.AluOpType.mult)
        nc.vector.tensor_tensor(xb, xb, cob[:], mybir.AluOpType.mult)
        nc.vector.tensor_tensor(xb, xb, t2[:], mybir.AluOpType.add)

    def up_proj(tc, wp, w_sb, kt, rhs_tiles, out_tile, h_cols, n, ev):
        # out_tile[:, nb] = w_sb[:, h_cols]^T @ rhs, contraction kt tiles
        ps = upp[0].tile([P, 512], FP, name="psu", tag="psu")
        for k in range(kt):
            mm(ps[:], w_sb[k][:, h_cols], rhs_tiles[k][:, n * 512:(n + 1) * 512],
                             start=(k == 0), stop=(k == kt - 1))
        if ev == 0:
            nc.scalar.copy(out_tile[:, n * 512:(n + 1) * 512], ps[:])
        else:
            nc.vector.tensor_copy(out_tile[:, n * 512:(n + 1) * 512], ps[:])

    with tile.TileContext(nc) as tc:
        with tc.tile_pool(name="kvout", bufs=1, side="right") as kvp:
            k_nope = [kvp.tile([P, S], R, name=f"kn{h}", tag=f"kn{h}")
                      for h in range(HPC)]
            k_rope = [kvp.tile([P, S], R, name=f"kr{p}", tag=f"kr{p}")
                      for p in range(HPC // 2)]
            v4 = [kvp.tile([P, HPC * DV], BF, name=f"v{t}", tag=f"v{t}")
                  for t in range(KT)]

            # ---- D2'/D1': sharded down-projections + all-gather ----
            # Core r of each 4-core batch group computes 1/4 of c_kv^T and
            # 1/4 of q_lat^T; both are all-gathered (DRAM bounce) while the
            # other partial / U2 runs on the PE.
            REP = [[0, 1, 2, 3], [4, 5, 6, 7]]
            dgp = tc.alloc_tile_pool(name="dgather", bufs=1, space="DRAM")
            ckv_in = dgp.tile([P, S], BF, name="ckv_in", tag="ckv_in")
            ckv_all = dgp.tile([KVL, S], BF, name="ckv_all", tag="ckv_all")
            ql_in = dgp.tile([QL // 4, S], BF, name="ql_in", tag="ql_in")
            ql_all = dgp.tile([QL, S], BF, name="ql_all", tag="ql_all")
            dpp = tc.alloc_tile_pool(name="dpart", bufs=1)
            ckv_part = dpp.tile([P, S], BF, name="ckv_part", tag="ckv_part")
            ql_part = [dpp.tile([P, S], BF, name="qlp0", tag="qlp0"),
                       dpp.tile([64, S], BF, name="qlp1", tag="qlp1")]

            down_part(tc, w_kvd_s, [P], [ckv_part], None)
            nc.sync.dma_start(out=ckv_in[:], in_=ckv_part[:])
            nc.gpsimd.collective_compute(
                "AllGather", mybir.AluOpType.bypass, replica_groups=REP,
                ins=[ckv_in[:].opt()], outs=[ckv_all[:].opt()])
            down_part(tc, w_qd_s, [P, 64], ql_part, SCALE)
            nc.sync.dma_start(out=ql_in[0:P], in_=ql_part[0][:])
            nc.sync.dma_start(out=ql_in[P:P + 64], in_=ql_part[1][:])
            nc.gpsimd.collective_compute(
                "AllGather", mybir.AluOpType.bypass, replica_groups=REP,
                ins=[ql_in[:].opt()], outs=[ql_all[:].opt()])
            dpp.release()

            cp_ = tc.alloc_tile_pool(name="ckvp", bufs=1)
            if True:
                c_kv = [cp_.tile([P, S], BF, name=f"ckv{m}", tag=f"ckv{m}")
                        for m in range(KVT)]
                # scalar hwdge queue: a reload parked on the CC semaphore
                # must not block the weight prefetches on the sync queue.
                # n-major chunks: U2's first column block only waits for the
                # first 4 chunks instead of the whole gather.
                for n in range(NB):
                    for m in range(KVT):
                        nc.scalar.dma_start(
                            out=c_kv[m][:, n * 512:(n + 1) * 512],
                            in_=ckv_all[m * P:(m + 1) * P,
                                        n * 512:(n + 1) * 512])

                # ---- U2: k-ups + rope-k + V4 ----
                with tc.tile_pool(name="wku", bufs=1) as wp, \
                     tc.tile_pool(name="csp2", bufs=4) as csp, \
                     tc.tile_pool(name="tp2", bufs=2) as tp, \
                     tc.tile_pool(name="psu2", bufs=3, space="PSUM") as pu:
                    upp = [pu]
                    ku = [wp.tile([P, HPC * DN], BF, name=f"wku{k}", tag=f"wku{k}")
                          for k in range(KVT)]
                    kr = [wp.tile([P, HPC * DR], BF, name=f"wkr{k}", tag=f"wkr{k}")
                          for k in range(KVT)]
                    vu = [wp.tile([P, HPC * DV], BF, name=f"wvu{k}", tag=f"wvu{k}")
                          for k in range(KVT)]
                    for k in range(KVT):
                        nc.sync.dma_start(out=ku[k][:], in_=w_ku[k * P:(k + 1) * P, :])
                        nc.sync.dma_start(out=kr[k][:], in_=w_kr[k * P:(k + 1) * P, :])
                        nc.sync.dma_start(out=vu[k][:], in_=w_vu[k * P:(k + 1) * P, :])
                    for n in range(NB):
                        for h in range(HPC):
                            up_proj(tc, pu, ku, KVT, c_kv, k_nope[h],
                                    slice(h * DN, (h + 1) * DN), n, h % 2)
                        for p in range(HPC // 2):
                            up_proj(tc, pu, kr, KVT, c_kv, k_rope[p],
                                    slice(p * 2 * DR, (p + 1) * 2 * DR), n, p % 2)
                            rope_block(tc, k_rope[p], csp, tp, n)
                    for t in range(KT):
                        ps = pu.tile([P, HPC * DV], FP, name="psv", tag="psv")
                        for k in range(KVT):
                            mm(ps[:], c_kv[k][:, t * P:(t + 1) * P],
                                             vu[k][:], start=(k == 0),
                                             stop=(k == KVT - 1))
                        if t % 2 == 0:
                            nc.scalar.copy(v4[t][:], ps[:])
                        else:
                            nc.vector.tensor_copy(v4[t][:], ps[:])

            # ---- D1 reload: gathered q_lat^T (already scaled) ----
            cp_.release()
            qlp = tc.alloc_tile_pool(name="qlatp", bufs=1)
            q_lat = [qlp.tile([P, S], BF, name=f"ql{m}", tag=f"ql{m}")
                     for m in range(QLT)]
            # sync queue: it is idle during U2, so these issue as soon as
            # the q_lat gather lands instead of queueing behind U2's evicts.
            for n in range(NB):
                for m in range(QLT):
                    nc.sync.dma_start(
                        out=q_lat[m][:, n * 512:(n + 1) * 512],
                        in_=ql_all[m * P:(m + 1) * P, n * 512:(n + 1) * 512])

            with tc.tile_pool(name="qout", bufs=1, side="right") as qp:
                q_nope = [qp.tile([P, S], R, name=f"qn{h}", tag=f"qn{h}")
                          for h in range(HPC)]
                q_rope = [qp.tile([P, S], R, name=f"qr{p}", tag=f"qr{p}")
                          for p in range(HPC // 2)]

                # ---- U1: q-ups + rope-q ----
                with tc.tile_pool(name="wqup", bufs=1) as wp, \
                     tc.tile_pool(name="csp1", bufs=2) as csp, \
                     tc.tile_pool(name="tp1", bufs=1) as tp, \
                     tc.tile_pool(name="psu1", bufs=4, space="PSUM") as pu:
                    upp = [pu]
                    wu = [wp.tile([P, HPC * DN], BF, name=f"wqu{k}", tag=f"wqu{k}")
                          for k in range(QLT)]
                    wr = [wp.tile([P, HPC * DR], BF, name=f"wqr{k}", tag=f"wqr{k}")
                          for k in range(QLT)]
                    for k in range(QLT):
                        nc.sync.dma_start(out=wu[k][:], in_=w_qu[k * P:(k + 1) * P, :])
                        nc.sync.dma_start(out=wr[k][:], in_=w_qr[k * P:(k + 1) * P, :])
                    for n in range(NB):
                        for h in range(HPC):
                            up_proj(tc, pu, wu, QLT, q_lat, q_nope[h],
                                    slice(h * DN, (h + 1) * DN), n, h % 2)
                        for p in range(HPC // 2):
                            up_proj(tc, pu, wr, QLT, q_lat, q_rope[p],
                                    slice(p * 2 * DR, (p + 1) * 2 * DR), n, p % 2)
                            rope_block(tc, q_rope[p], csp, tp, n)
                qlp.release()

                # ---- ATT + o_proj per q-block ----
                with tc.tile_pool(name="att_c", bufs=1) as cp, \
                     tc.tile_pool(name="dnsb", bufs=2) as dnp, \
                     tc.tile_pool(name="probs", bufs=4) as prp, \
                     tc.tile_pool(name="attn", bufs=5) as atp, \
                     tc.tile_pool(name="osb", bufs=2) as osp, \
                     tc.tile_pool(name="rdn", bufs=2) as rdp, \
                     tc.tile_pool(name="rbp", bufs=2) as rbp, \
                     tc.tile_pool(name="ps_s", bufs=3, space="PSUM") as ps_s, \
                     tc.tile_pool(name="ps_pv", bufs=2, space="PSUM") as ps_pv, \
                     tc.tile_pool(name="ps_den", bufs=2, space="PSUM") as ps_den, \
                     tc.tile_pool(name="ps_o", bufs=1, space="PSUM") as ps_o:
                    wo = [cp.tile([P, HID], BF, name=f"wo{k}", tag=f"wo{k}")
                          for k in range(HPC)]
                    for k in range(HPC):
                        nc.sync.dma_start(out=wo[k][:], in_=w_o[k * P:(k + 1) * P, :])
                    ones_k = cp.tile([P, 2], BF, name="ones_k", tag="ones_k")
                    nc.vector.memset(ones_k[:], 1.0)
                    if variant == "causal":
                        msk = cp.tile([P, 896], FP, name="msk", tag="msk")
                        nc.gpsimd.memset(msk[:], 0.0)
                        nc.gpsimd.affine_select(
                            out=msk[:], in_=msk[:],
                            compare_op=mybir.AluOpType.is_ge,
                            fill=NEG, base=-384,
                            pattern=[[1, 896]], channel_multiplier=-1)
                    if variant == "generic":
                        mrp = tc.alloc_tile_pool(name="mrhs", bufs=18)

                    # Scores are pipelined through a flat emission cursor that
                    # stays 2 tiles ahead of consumption, across head and
                    # q-block boundaries: while the softmax tail (den matmul ->
                    # reciprocal -> broadcast -> normalize) and o_proj of one
                    # block drain, the PE is already streaming the next head's
                    # score matmuls.
                    def nkt_of(jj):
                        return 4 * (jj + 1) if variant == "causal" else KT

                    seq = [(jj, h, ki) for jj in range(NB) for h in range(HPC)
                           for ki in range(nkt_of(jj))]
                    pos_of = {t: i for i, t in enumerate(seq)}

                    def emit_ss(jj, h, ki):
                        pp_, hh = h // 2, (h % 2) * DR
                        ss = ps_s.tile([P, 512], FP)
                        mm(ss[:], k_nope[h][:, ki * P:(ki + 1) * P],
                           q_nope[h][:, jj * 512:(jj + 1) * 512],
                           start=True, stop=False)
                        mm(ss[:],
                           k_rope[pp_][hh:hh + DR, ki * P:(ki + 1) * P],
                           q_rope[pp_][hh:hh + DR, jj * 512:(jj + 1) * 512],
                           start=False, stop=True)
                        return ss

                    emitted = {}
                    cursor = [0]

                    def ensure_ss(idx):
                        while cursor[0] <= min(idx, len(seq) - 1):
                            t = seq[cursor[0]]
                            emitted[t] = emit_ss(*t)
                            cursor[0] += 1

                    for j in range(NB):
                        nkt = nkt_of(j)
                        mts = []
                        if variant == "generic":
                            for ki in range(KT):
                                mt = mrp.tile([P, 512], FP, name="mrhs", tag="mrhs")
                                nc.sync.dma_start(
                                    out=mt[:],
                                    in_=maskT[ki * P:(ki + 1) * P,
                                              j * 512:(j + 1) * 512])
                                mts.append(mt)
                        attn_sb = []
                        for h in range(HPC):
                            pv = ps_pv.tile([P, 512], FP)
                            den_acc = dnp.tile([P, 512], BF, name="den_acc",
                                               tag="den_acc")
                            for ki in range(nkt):
                                ensure_ss(pos_of[(j, h, ki)] + 2)
                                ss = emitted.pop((j, h, ki))
                                pr = prp.tile([P, 512], BF, name="pr", tag="pr")
                                off = 128 * ki - 512 * j
                                if variant == "causal" and off >= 0:
                                    c0 = 384 - off
                                    nc.vector.tensor_tensor(
                                        ss[:], ss[:], msk[:, c0:c0 + 512],
                                        mybir.AluOpType.add)
                                elif variant == "generic":
                                    nc.vector.tensor_tensor(
                                        ss[:], ss[:], mts[ki][:],
                                        mybir.AluOpType.add)
                                nc.scalar.activation(
                                    pr[:], ss[:],
                                    mybir.ActivationFunctionType.Exp)
                                mm(pv[:], v4[ki][:, h * DV:(h + 1) * DV],
                                   pr[:], start=(ki == 0),
                                   stop=(ki == nkt - 1))
                                if ki == 0:
                                    nc.vector.tensor_copy(den_acc[:], pr[:])
                                else:
                                    nc.vector.tensor_tensor(
                                        den_acc[:], den_acc[:], pr[:],
                                        mybir.AluOpType.add)
                            den = ps_den.tile([2, 512], FP)
                            mm(den[:], ones_k[:], den_acc[:],
                               start=True, stop=True)
                            rden = rdp.tile([1, 512], FP, name="rden",
                                            tag="rden")
                            nc.vector.reciprocal(rden[:], den[0:1])
                            rb = rbp.tile([P, 512], FP, name="rb", tag="rb")
                            nc.gpsimd.partition_broadcast(rb[:], rden[:])
                            at = atp.tile([P, 512], BF, name="at", tag="at")
                            nc.vector.tensor_tensor(at[:], pv[:], rb[:],
                                                    mybir.AluOpType.mult)
                            attn_sb.append(at)
                        for t in range(4):
                            ob = osp.tile([P, HID], FP, name="ob", tag="ob")
                            for nn in range(NB):
                                po = ps_o.tile([P, 512], FP)
                                for kk in range(HPC):
                                    mm(
                                        po[:], attn_sb[kk][:, t * P:(t + 1) * P],
                                        wo[kk][:, nn * 512:(nn + 1) * 512],
                                        start=(kk == 0), stop=(kk == HPC - 1))
                                if nn % 2 == 0:
                                    nc.scalar.copy(ob[:, nn * 512:(nn + 1) * 512],
                                                   po[:])
                                else:
                                    nc.vector.tensor_copy(
                                        ob[:, nn * 512:(nn + 1) * 512], po[:])
                            nc.sync.dma_start(
                                out=o_out[(j * 4 + t) * P:(j * 4 + t + 1) * P, :],
                                in_=ob[:])
                    if variant == "generic":
                        mrp.release()

    nc.compile()
    return nc


def _get(variant):
    if variant not in _cache:
        _cache[variant] = _build(variant)
    return _cache[variant]


def _host_prep(inputs):
    hs = np.ascontiguousarray(inputs["hidden_states"], dtype=np.float32)
    mask = np.asarray(inputs["attention_mask"], dtype=np.float32)
    pos = np.asarray(inputs["position_ids"])
    B = hs.shape[0]

    causal = np.where(np.tril(np.ones((S, S), dtype=bool)), np.float32(0.0),
                      np.float32(NEG))
    variant = "causal"
    for b in range(B):
        if not np.array_equal(mask[b, 0], causal):
            variant = "zeros" if not mask.any() else "generic"
            break

    inv_freq = (1.0 / (ROPE_BASE ** (np.arange(0, DR, 2, dtype=np.float32) / DR)))
    css = []
    for b in range(B):
        t = pos[b].astype(np.float32)
        freqs = t[:, None] * inv_freq[None, :]  # [S, 32]
        cf = np.cos(freqs).T  # [32, S]
        sf = np.sin(freqs).T
        cs = np.empty((128, 2 * S), dtype=np.float32)
        for q in range(4):
            cs[q * 32:(q + 1) * 32, :S] = cf
            cs[q * 32:(q + 1) * 32, S:] = sf if q % 2 else -sf
        css.append(np.ascontiguousarray(cs))
    return hs, mask, css, variant


def kernel(**inputs):
    hs, mask, css, variant = _host_prep(inputs)
    nc = _get(variant)

    w_qd = np.asarray(inputs["W_q_down"], dtype=np.float32)
    w_kvd = np.asarray(inputs["W_kv_down"], dtype=np.float32)
    W_qu = np.asarray(inputs["W_q_up"], dtype=np.float32)
    W_qr = np.asarray(inputs["W_q_rope"], dtype=np.float32)
    W_ku = np.asarray(inputs["W_k_up"], dtype=np.float32)
    W_kr = np.asarray(inputs["W_k_rope"], dtype=np.float32)
    W_vu = np.asarray(inputs["W_v_up"], dtype=np.float32)
    W_o = np.asarray(inputs["W_o"], dtype=np.float32)

    hidT = [np.ascontiguousarray(hs[b].T).astype(ml_dtypes.bfloat16)
            for b in range(2)]
    maskT = [np.ascontiguousarray(mask[b, 0].T) for b in range(2)] \
        if variant == "generic" else None

    in_maps = []
    for core in range(NCORES):
        b, hg = divmod(core, NCORES // 2)
        m = {
            "hidT": hidT[b],
            "w_qd_s": np.ascontiguousarray(
                w_qd[:, hg * (QL // 4):(hg + 1) * (QL // 4)]).astype(
                    ml_dtypes.bfloat16),
            "w_kvd_s": np.ascontiguousarray(
                w_kvd[:, hg * (KVL // 4):(hg + 1) * (KVL // 4)]).astype(
                    ml_dtypes.bfloat16),
            "w_qu": np.ascontiguousarray(
                W_qu[:, hg * HPC * DN:(hg + 1) * HPC * DN]).astype(
                    ml_dtypes.bfloat16),
            "w_qr": np.ascontiguousarray(
                W_qr[:, hg * HPC * DR:(hg + 1) * HPC * DR]).astype(
                    ml_dtypes.bfloat16),
            "w_ku": np.ascontiguousarray(
                W_ku[:, hg * HPC * DN:(hg + 1) * HPC * DN]).astype(
                    ml_dtypes.bfloat16),
            "w_kr": np.ascontiguousarray(
                W_kr[:, hg * HPC * DR:(hg + 1) * HPC * DR]).astype(
                    ml_dtypes.bfloat16),
            "w_vu": np.ascontiguousarray(
                W_vu[:, hg * HPC * DV:(hg + 1) * HPC * DV]).astype(
                    ml_dtypes.bfloat16),
            "w_o": np.ascontiguousarray(
                W_o[hg * HPC * DV:(hg + 1) * HPC * DV, :]).astype(
                    ml_dtypes.bfloat16),
            "cs": css[b],
        }
        if maskT is not None:
            m["maskT"] = maskT[b]
        in_maps.append(m)

    global _last_in_maps, _last_nc
    _last_in_maps, _last_nc = in_maps, nc
    res = run_bass_kernel_spmd(nc, in_maps, core_ids=list(range(NCORES)))
    out = np.zeros((2, S, HID), dtype=np.float32)
    for core in range(NCORES):
        b = core // (NCORES // 2)
        out[b] += res.results[core]["o"]
    return out

